# revision 46
# baseline (speedup 1.0000x reference)
"""Document-causal GQA attention on 8 TRN2 NeuronCores.

Strategy: the packed-document mask makes attention block-diagonal over
(batch, document) segments, so each of the 8 cores gets one segment's
queries (2 batches x ~4 docs) together with its KV window — no
cross-core communication at all. The host shards/transposes inputs,
each core runs the full QKV->RoPE->softmax->PV->Wo pipeline on its
rows, and the host scatters the disjoint output rows back.

Fast path (single-doc segments, zero key offset, NQ=640): K and Q
share one x/rope buffer, QK scores are packed into four 512-wide PSUM
tiles so the ACT engine runs one wide exp per tile, diagonal-block
masks are packed contiguously and multiplied on the idle GpSimd
engine, softmax denominators use the fast approximate reciprocal, and
the output projection is split into j-rounds that run interleaved with
late attention tiles (bf16 partial sums in SBUF) so the Tensor engine
never drains.
"""
import numpy as np
import ml_dtypes

from contextlib import ExitStack

import concourse.bass as bass
import concourse.tile as tile
from concourse import bacc, mybir
from concourse.bass_utils import run_bass_kernel_spmd

BS, S, D, H, KVH, HD = 2, 2048, 2048, 32, 8, 64
N_REP = H // KVH
HQ = H * HD
HKV = KVH * HD
P = 128
N_CORES = 8
DT = D // P
HC = HQ // 512
HQT = HQ // P

f32 = mybir.dt.float32
bf16 = mybir.dt.bfloat16
EXPF = mybir.ActivationFunctionType.Exp
bf = ml_dtypes.bfloat16

HEAD_ORDER = [i // 2 if i % 2 == 0 else 16 + i // 2 for i in range(32)]
# diagonal-block packing order for the fast path (see _build_causal)
DIAG_ORDER = [0, 3, 1, 4, 2]


# ---------------------------------------------------------------------------
# host-side planning
# ---------------------------------------------------------------------------

def _round_up(x, m):
    return ((x + m - 1) // m) * m


def _plan_jobs(sequence_id):
    jobs = []
    for b in range(BS):
        sid = np.asarray(sequence_id[b])
        starts = [0] + list(np.where(np.diff(sid) != 0)[0] + 1) + [len(sid)]
        for i in range(len(starts) - 1):
            jobs.append([b, int(starts[i]), int(starts[i + 1] - starts[i]),
                         int(starts[i])])
    while len(jobs) > N_CORES:
        best, bi = None, -1
        for i in range(len(jobs) - 1):
            a, c = jobs[i], jobs[i + 1]
            if a[0] == c[0] and a[1] + a[2] == c[1]:
                cost = (c[1] + c[2]) - min(a[3], c[3])
                if best is None or cost < best:
                    best, bi = cost, i
        a, c = jobs[bi], jobs[bi + 1]
        jobs[bi] = [a[0], a[1], a[2] + c[2], min(a[3], c[3])]
        del jobs[bi + 1]
    while len(jobs) < N_CORES:
        i = max(range(len(jobs)), key=lambda j: jobs[j][2])
        b, qs, ql, ks = jobs[i]
        if ql < 2:
            jobs.append([b, qs, 0, qs])
            continue
        h = ql // 2
        jobs[i] = [b, qs, h, ks]
        jobs.insert(i + 1, [b, qs + h, ql - h, ks])
    return jobs


def _permute_wq(wq_t):
    return np.ascontiguousarray(
        wq_t.reshape(D, 32, 64)[:, HEAD_ORDER, :].reshape(D, HQ))


def _permute_wo(wo_t):
    return np.ascontiguousarray(
        wo_t.reshape(32, 64, D)[HEAD_ORDER].reshape(HQ, D))


def _rope_tabs(start, ln, n, cos_tab, sin_tab):
    cos = np.ones((n, 64), dtype=np.float32)
    sin = np.zeros((n, 64), dtype=np.float32)
    c = cos_tab[start:start + ln]
    s = sin_tab[start:start + ln]
    cos[:ln, 0::2] = c
    cos[:ln, 1::2] = c
    sin[:ln, 0::2] = -s
    sin[:ln, 1::2] = s
    return (np.tile(cos, (1, 8)).astype(bf), np.tile(sin, (1, 8)).astype(bf))


def _seg_mask(job, NQ, NK, sequence_id):
    b, qs, ql, ks = job
    kl = qs + ql - ks
    sid = np.asarray(sequence_id[b])
    sid_q = np.full(NQ, -2, dtype=np.int64)
    sid_q[:ql] = sid[qs:qs + ql]
    sid_k = np.full(NK, -1, dtype=np.int64)
    sid_k[:kl] = sid[ks:ks + kl]
    gq = qs + np.arange(NQ)
    gk = ks + np.arange(NK)
    mask = ((sid_k[:, None] == sid_q[None, :]) &
            (gk[:, None] <= gq[None, :])).astype(np.float32)
    # padded query columns attend to key 0 so denominators stay finite
    mask[0, ql:] = 1.0
    return mask, kl


def _pmajor(a, np_, inner):
    """[(t p), c] -> [p, t*c] so DMA lines per partition are t-contiguous."""
    t = a.shape[0] // np_
    return np.ascontiguousarray(
        a.reshape(t, np_, inner).transpose(1, 0, 2).reshape(np_, t * inner))


def _core_inputs_causal(job, NQ, x, sequence_id, cos_tab, sin_tab):
    b, qs, ql, ks = job
    x_t = np.zeros((D, NQ), dtype=np.float32)
    x_t[:, :ql] = x[b, qs:qs + ql].T
    cos_q, sin_q = _rope_tabs(qs, ql, NQ, cos_tab, sin_tab)
    mask, _ = _seg_mask(job, NQ, NQ, sequence_id)
    NQT = NQ // P
    # diag-block masks, packed: cols 0:640 additive (0 / -2400, added into
    # PSUM before exp), cols 640:1280 multiplicative (1 / 0, applied after)
    maskp = np.zeros((P, 2 * NQT * P), dtype=np.float32)
    for pos, kt in enumerate(DIAG_ORDER[:NQT]):
        blk = mask[kt * P:(kt + 1) * P, kt * P:(kt + 1) * P]
        maskp[:, pos * P:(pos + 1) * P] = -2400.0 * (1.0 - blk)
        maskp[:, NQT * P + pos * P:NQT * P + (pos + 1) * P] = blk
    kones = np.zeros((NQ, 1), dtype=np.float32)
    kones[:ql] = 1.0
    return {
        "x_t": _pmajor(x_t.astype(bf), P, NQ),
        "cos_t": _pmajor(cos_q, P, 512), "sin_t": _pmajor(sin_q, P, 512),
        "maskp": maskp.astype(bf), "kones": kones.astype(bf),
    }


def _core_inputs_general(job, NQ, NK, x, sequence_id, cos_tab, sin_tab):
    b, qs, ql, ks = job
    kl = qs + ql - ks

    xq_t = np.zeros((D, NQ), dtype=np.float32)
    xq_t[:, :ql] = x[b, qs:qs + ql].T
    xk_t = np.zeros((D, NK), dtype=np.float32)
    xk_t[:, :kl] = x[b, ks:ks + kl].T

    cos_q, sin_q = _rope_tabs(qs, ql, NQ, cos_tab, sin_tab)
    cos_k, sin_k = _rope_tabs(ks, kl, NK, cos_tab, sin_tab)

    mask, kl = _seg_mask(job, NQ, NK, sequence_id)
    kones = np.zeros((NK, 1), dtype=np.float32)
    kones[:kl] = 1.0

    return {
        "xq_t": xq_t.astype(bf), "xk_t": xk_t.astype(bf),
        "cos_q": cos_q, "sin_q": sin_q, "cos_k": cos_k, "sin_k": sin_k,
        "maskm": mask.astype(bf), "kones": kones.astype(bf),
    }


# ---------------------------------------------------------------------------
# fast-path device graph (single-doc causal segments, NQ == NK == 640)
# ---------------------------------------------------------------------------

_BUILD_CACHE = {}


def _build_causal(NQ):
    key = ("causal", NQ)
    if key in _BUILD_CACHE:
        return _BUILD_CACHE[key]
    NQT = NQ // P
    assert NQT == 5

    nc = bacc.Bacc("TRN2", target_bir_lowering=False, debug=False,
                   num_devices=N_CORES)

    # p-major DRAM layouts (long contiguous per-partition DMA lines)
    x_d = nc.dram_tensor("x_t", [P, DT * NQ], bf16, kind="ExternalInput").ap()
    wq_d = nc.dram_tensor("wq_t", [P, HC * DT * 512], bf16,
                          kind="ExternalInput").ap()
    wk_d = nc.dram_tensor("wk_t", [P, DT * 512], bf16,
                          kind="ExternalInput").ap()
    wv_d = nc.dram_tensor("wv_t", [P, DT * 512], bf16,
                          kind="ExternalInput").ap()
    wo_d = nc.dram_tensor("wo_t", [P, 4 * HQT * 512], bf16,
                          kind="ExternalInput").ap()
    cos_d = nc.dram_tensor("cos_t", [P, NQT * 512], bf16,
                           kind="ExternalInput").ap()
    sin_d = nc.dram_tensor("sin_t", [P, NQT * 512], bf16,
                           kind="ExternalInput").ap()
    maskp_d = nc.dram_tensor("maskp", [P, 2 * NQT * P], bf16,
                             kind="ExternalInput").ap()
    kones_d = nc.dram_tensor("kones", [NQ, 1], bf16, kind="ExternalInput").ap()
    id_d = nc.dram_tensor("ident", [P, P], bf16, kind="ExternalInput").ap()
    out_d = nc.dram_tensor("out", [NQ, HQ], bf16, kind="ExternalOutput").ap()
    rsd = nc.dram_tensor("rsd", [P, 8 * NQ], bf16)
    DEBUG = False
    if DEBUG:
        attn_dbg = nc.dram_tensor("attn_dbg", [P, HQT * NQ], bf16,
                                  kind="ExternalOutput").ap()
        part_dbg = nc.dram_tensor("part_dbg", [P, 20 * 512], bf16,
                                  kind="ExternalOutput").ap()
        rs_dbg = nc.dram_tensor("rs_dbg", [P, NQ], f32,
                                kind="ExternalOutput").ap()

    with tile.TileContext(nc) as tc, ExitStack() as ctx:
        const = ctx.enter_context(tc.tile_pool(name="const", bufs=1))
        persist = ctx.enter_context(tc.tile_pool(name="persist", bufs=1))
        xpool = ctx.enter_context(tc.tile_pool(name="xpool", bufs=1))
        wstream = ctx.enter_context(tc.tile_pool(name="wstream", bufs=2))
        work = ctx.enter_context(tc.tile_pool(name="work", bufs=2))
        ropetab = ctx.enter_context(tc.tile_pool(name="ropetab", bufs=1))
        pmp = ctx.enter_context(tc.tile_pool(name="pmp", bufs=3))
        rbpool = ctx.enter_context(tc.tile_pool(name="rbpool", bufs=4))
        # PSUM: psS 4 banks, psOm 2 banks, woacc 2 banks = 8
        psS = ctx.enter_context(tc.tile_pool(name="psS", bufs=4, space="PSUM"))
        psOm = ctx.enter_context(tc.tile_pool(name="psOm", bufs=2,
                                              space="PSUM"))
        woacc = ctx.enter_context(tc.tile_pool(name="woacc", bufs=2,
                                               space="PSUM"))

        # ---- initial loads: x/wk chunks first so the PE starts ASAP ----
        x_sb = xpool.tile([P, DT, NQ], bf16, name="x_sb")
        x_r = x_d.rearrange("p (t q) -> p t q", t=DT)
        wvc = wstream.tile([P, DT, 512], bf16, name="wchunk")
        wv_r = wv_d.rearrange("p (t o) -> p t o", t=DT)
        wkc = wstream.tile([P, DT, 512], bf16, name="wchunk")
        wk_r = wk_d.rearrange("p (t o) -> p t o", t=DT)
        for a, b2 in [(0, 1), (1, 2), (2, 3), (3, 4), (4, 6), (6, 8),
                      (8, 12), (12, 16)]:
            nc.sync.dma_start(x_sb[:, a:b2, :], x_r[:, a:b2, :])
            nc.scalar.dma_start(wvc[:, a:b2, :], wv_r[:, a:b2, :])
        nc.gpsimd.dma_start(wkc[:], wk_r)

        ident = const.tile([P, P], bf16, name="ident")
        nc.gpsimd.dma_start(ident[:], id_d)

        Qt = persist.tile([P, HQT, NQ], bf16, name="Qt")
        KtRz = persist.tile([P, KVH, NQ], bf16, name="KtRz")
        Vaug = persist.tile([P, NQT, KVH, P], bf16, name="Vaug")
        attnT = persist.tile([P, HQT, NQ], bf16, name="attnT")
        maskp = persist.tile([P, 2 * NQT * P], bf16, name="maskp")
        partial = persist.tile([P, 20, 512], bf16, name="partial")
        # softmax denominators: quarter index on the free axis so the custom
        # reciprocal op always runs at partition offset 0
        rs_all = persist.tile([P, 8, NQ], f32, name="rs_all")

        nc.vector.memset(KtRz[64:128, 0:4, :], 0.0)
        nc.vector.memset(KtRz[0:64, 4:8, :], 0.0)
        nc.vector.memset(Vaug[:, :, :, HD:P], 0.0)
        kones_sb = const.tile([P, NQT], bf16, name="kones_sb")
        nc.gpsimd.dma_start(kones_sb[:],
                            kones_d.rearrange("(t p) o -> p (t o)", p=P))
        for kt in range(NQT):
            for g in range(KVH):
                nc.vector.tensor_copy(Vaug[:, kt, g, HD:HD + 1],
                                      kones_sb[:, kt:kt + 1])

        cost = ropetab.tile([P, NQT, 512], bf16, name="cost")
        sint = ropetab.tile([P, NQT, 512], bf16, name="sint")
        cos_r = cos_d.rearrange("p (t c) -> p t c", t=NQT)
        sin_r = sin_d.rearrange("p (t c) -> p t c", t=NQT)
        nc.scalar.dma_start(cost[:, 0:2, :], cos_r[:, 0:2, :])
        nc.scalar.dma_start(sint[:, 0:2, :], sin_r[:, 0:2, :])
        nc.scalar.dma_start(cost[:, 2:NQT, :], cos_r[:, 2:NQT, :])
        nc.scalar.dma_start(sint[:, 2:NQT, :], sin_r[:, 2:NQT, :])
        nc.gpsimd.dma_start(maskp[:], maskp_d)

        def rope_block(ps, ti):
            nat = work.tile([P, 512], bf16, name="nat")
            nc.vector.tensor_copy(nat[:], ps[:])
            ro = work.tile([P, 512], bf16, name="ro")
            nc.gpsimd.tensor_mul(ro[:, 0::2], nat[:, 1::2], sint[:, ti, 0::2])
            nc.gpsimd.tensor_mul(ro[:, 1::2], nat[:, 0::2], sint[:, ti, 1::2])
            tmp = work.tile([P, 512], bf16, name="tmp")
            nc.vector.tensor_mul(tmp[:], nat[:], cost[:, ti, :])
            rot = work.tile([P, 512], bf16, name="rot")
            nc.vector.tensor_add(rot[:], ro[:], tmp[:])
            return rot

        # ---- K projection + rope + transpose (zero-padded halves) ----
        def k_evict(ps, kt):
            rot = rope_block(ps, kt)
            ks = slice(kt * P, (kt + 1) * P)
            for b in range(4):
                pool = psOm if b % 2 == 0 else woacc
                pst = pool.tile([P, P], bf16, name="po")
                nc.tensor.transpose(pst[:], rot[:, b * P:(b + 1) * P], ident[:])
                half = (2 * b) // 4
                lo = half * 64
                nc.scalar.copy(KtRz[lo:lo + 64, 2 * b, ks], pst[0:64, :])
                nc.scalar.copy(KtRz[lo:lo + 64, 2 * b + 1, ks], pst[64:128, :])

        # ---- V projection first (dt-outer; eviction is a cheap copy) ----
        vacc = [psS.tile([P, 512], f32, name="ps") for _ in range(4)]
        vacc.append(psOm.tile([P, 512], f32, name="po"))
        for dt in range(DT):
            for kt in range(NQT):
                nc.tensor.matmul(vacc[kt][:],
                                 x_sb[:, dt, kt * P:(kt + 1) * P],
                                 wvc[:, dt, :], start=(dt == 0),
                                 stop=(dt == DT - 1))
        for kt in range(NQT):
            nc.vector.tensor_copy(
                Vaug[:, kt, :, 0:HD],
                vacc[kt][:].rearrange("p (g d) -> p g d", g=KVH))

        # ---- K projection (kt-major, rope evictions one tile behind) ----
        prevk = None
        for kt in range(NQT):
            ps = psS.tile([P, 512], f32, name="ps")
            for dt in range(DT):
                nc.tensor.matmul(ps[:], x_sb[:, dt, kt * P:(kt + 1) * P],
                                 wkc[:, dt, :], start=(dt == 0),
                                 stop=(dt == DT - 1))
            if prevk is not None:
                k_evict(*prevk)
            prevk = (ps, kt)
        k_evict(*prevk)

        # ---- Q projection + rope + transpose (head-permuted wq) ----
        def q_evict(ps, hc, qt):
            rot = rope_block(ps, qt)
            for b in range(4):
                pool = psOm if b % 2 == 0 else woacc
                pst = pool.tile([P, P], bf16, name="po")
                nc.tensor.transpose(pst[:], rot[:, b * P:(b + 1) * P], ident[:])
                dst = Qt[:, hc * 4 + b, qt * P:(qt + 1) * P]
                if b % 2 == 0:
                    nc.scalar.copy(dst, pst[:])
                else:
                    nc.vector.tensor_copy(dst, pst[:])

        wq_r = wq_d.rearrange("p (h t c) -> p h t c", h=HC, t=DT)
        prevq = None
        for hc in range(HC):
            wqc = wstream.tile([P, DT, 512], bf16, name="wchunk")
            nc.sync.dma_start(wqc[:], wq_r[:, hc:hc + 1, :, :].squeeze(1))
            for qt in range(NQT):
                ps = psS.tile([P, 512], f32, name="ps")
                for dt in range(DT):
                    nc.tensor.matmul(ps[:], x_sb[:, dt, qt * P:(qt + 1) * P],
                                     wqc[:, dt, :], start=(dt == 0),
                                     stop=(dt == DT - 1))
                if prevq is not None:
                    q_evict(*prevq)
                prevq = (ps, hc, qt)
        q_evict(*prevq)

        # ---- attention -------------------------------------------------
        # Packed score tiles per (t, par), q-columns grouped so each 512-wide
        # PSUM tile gets one exp and diagonal blocks sit contiguously:
        #   T1 = [kt0 q0:512]
        #   T2 = [kt3 q384:512 | kt1 q128:512]
        #   T3 = [kt4 q512:640 | kt2 q256:512 | kt0 q512:640]
        #   T4 = [kt1 q512:640 | kt2 q512:640 | kt3 q512:640]
        # diag masks: pm1[0:128]*mp[0:128], pm2[0:256]*mp[128:384],
        #             pm3[0:256]*mp[384:640]
        # QK matmuls run grouped by stationary K block (one LDWEIGHTS per
        # block); kt order 0,1,3,2,4 completes T1,T2,T4,T3 in that order so
        # the exp chain starts early.  For t < 8 masks are added into PSUM
        # via ident-matmuls (PE has slack); for t >= 8 masks are applied
        # multiplicatively on Vector/GpSimd and wo chunks fill the PE.
        MQ = NQT * P
        # (kt, [(T-index, dlo, dhi, qlo, qhi, last-write-of-T), ...])
        KT_GROUPS = [
            (0, [(0, 0, 128, 0, 128, False), (0, 128, 512, 128, 512, True),
                 (2, 384, 512, 512, 640, False)]),
            (1, [(1, 128, 256, 128, 256, False), (1, 256, 512, 256, 512, False),
                 (3, 0, 128, 512, 640, False)]),
            (3, [(1, 0, 128, 384, 512, True), (3, 256, 384, 512, 640, False)]),
            (2, [(2, 128, 256, 256, 384, False), (2, 256, 384, 384, 512, False),
                 (3, 128, 256, 512, 640, True)]),
            (4, [(2, 0, 128, 512, 640, True)]),
        ]

        def attn_tile(t, wo_fill):
            additive = True
            groups = (t // 4, 4 + t // 4)
            for par in range(2):
                gg = groups[par]
                h_lo = par * 64

                Ts = [psS.tile([P, 512], f32, name="ps") for _ in range(4)]
                # start=True only on each bank's first matmul: first_mm
                # clears the WHOLE bank's has_written bits, so any later
                # start=True would break accumulation onto the ident bias
                first = [True, True, True, True]

                def bias_mm(ti, dlo, dhi, mlo, mhi):
                    nc.tensor.matmul(Ts[ti][:, dlo:dhi], ident[:],
                                     maskp[:, mlo:mhi], start=first[ti],
                                     stop=False, skip_group_check=True)
                    first[ti] = False

                if additive:
                    bias_mm(0, 0, 128, 0, 128)
                for kt, mms in KT_GROUPS:
                    if additive and kt == 1:
                        bias_mm(1, 0, 256, 128, 384)
                        bias_mm(2, 0, 256, 384, 640)
                    for (ti, dlo, dhi, qlo, qhi, last) in mms:
                        nc.tensor.matmul(Ts[ti][:, dlo:dhi],
                                         KtRz[:, gg, kt * P:(kt + 1) * P],
                                         Qt[:, t, qlo:qhi],
                                         start=first[ti], stop=last,
                                         skip_group_check=True)
                        first[ti] = False

                pm1 = pmp.tile([P, 512], bf16, name="pm1")
                nc.scalar.activation(pm1[:], Ts[0][:, 0:512], EXPF,
                                     bias=0.0, scale=0.125)
                pm2 = pmp.tile([P, 512], bf16, name="pm2")
                nc.scalar.activation(pm2[:], Ts[1][:, 0:512], EXPF,
                                     bias=0.0, scale=0.125)
                pm4 = pmp.tile([P, 384], bf16, name="pm4")
                nc.scalar.activation(pm4[:], Ts[3][:, 0:384], EXPF,
                                     bias=0.0, scale=0.125)
                pm3 = pmp.tile([P, 512], bf16, name="pm3")
                nc.scalar.activation(pm3[:], Ts[2][:, 0:512], EXPF,
                                     bias=0.0, scale=0.125)
                if not additive:
                    nc.vector.tensor_mul(pm1[:, 0:128], pm1[:, 0:128],
                                         maskp[:, MQ:MQ + 128])
                    nc.vector.tensor_mul(pm2[:, 0:256], pm2[:, 0:256],
                                         maskp[:, MQ + 128:MQ + 384])
                    nc.gpsimd.tensor_mul(pm3[:, 0:256], pm3[:, 0:256],
                                         maskp[:, MQ + 384:MQ + 640])

                wo_fill()

                pO = psOm.tile([P, 512], f32, name="po")
                pT = woacc.tile([P, 128], f32, name="po")
                # V-stationary-grouped PV; pm3-dependent pieces last
                nc.tensor.matmul(pO[:, 0:512], Vaug[:, 0, gg, :],
                                 pm1[:, 0:512], start=True, stop=False,
                                 skip_group_check=True)
                nc.tensor.matmul(pO[:, 128:512], Vaug[:, 1, gg, :],
                                 pm2[:, 128:512], start=False, stop=False,
                                 skip_group_check=True)
                nc.tensor.matmul(pT[:], Vaug[:, 1, gg, :], pm4[:, 0:128],
                                 start=True, stop=False,
                                 skip_group_check=True)
                nc.tensor.matmul(pO[:, 384:512], Vaug[:, 3, gg, :],
                                 pm2[:, 0:128], start=False, stop=False,
                                 skip_group_check=True)
                nc.tensor.matmul(pT[:], Vaug[:, 3, gg, :], pm4[:, 256:384],
                                 start=False, stop=False,
                                 skip_group_check=True)
                nc.tensor.matmul(pO[:, 256:512], Vaug[:, 2, gg, :],
                                 pm3[:, 128:384], start=False, stop=True,
                                 skip_group_check=True)
                nc.tensor.matmul(pT[:], Vaug[:, 2, gg, :], pm4[:, 128:256],
                                 start=False, stop=False,
                                 skip_group_check=True)
                nc.tensor.matmul(pT[:], Vaug[:, 0, gg, :], pm3[:, 384:512],
                                 start=False, stop=False,
                                 skip_group_check=True)
                nc.tensor.matmul(pT[:], Vaug[:, 4, gg, :], pm3[:, 0:128],
                                 start=False, stop=True,
                                 skip_group_check=True)

                rsum = work.tile([1, 640], f32, name="rsum")
                nc.vector.tensor_copy(rsum[:, 0:512], pO[64:65, 0:512])
                nc.vector.tensor_copy(rsum[:, 512:640], pT[64:65, :])
                rq = (t % 2) * 2 + par
                nc.sync.dma_start(rs_all[rq:rq + 1, t // 2, :], rsum[:])
                nc.vector.tensor_copy(attnT[h_lo:h_lo + 64, t, 0:512],
                                      pO[0:64, :])
                nc.vector.tensor_copy(attnT[h_lo:h_lo + 64, t, 512:640],
                                      pT[0:64, :])

        DMA_ENGS = [nc.sync, nc.gpsimd]

        def norm_pair(t):
            # normalize attnT tiles (t-1, t) as soon as their rsums land;
            # rs rows are indexed by half-quarter so the reciprocal always
            # runs at partition offset 0
            fidx = t // 2
            late = t == HQT - 1
            rs_rcp = work.tile([4, NQ], f32, name="rs_rcp", bufs=1)
            rsb = work.tile([4, NQ], bf16, name="rsb", bufs=1)
            nc.vector.reciprocal_approx_fast(rs_rcp[0:4, :],
                                             rs_all[0:4, fidx, :])
            nc.vector.tensor_copy(rsb[0:4, :], rs_rcp[0:4, :])
            nc.sync.dma_start(
                rsd.ap()[0:4, fidx * NQ:(fidx + 1) * NQ],
                rsb[0:4, :])
            for t2 in (t - 1, t):
                for par in range(2):
                    h_lo = par * 64
                    rq = (t2 % 2) * 2 + par
                    rb = rbpool.tile([P, NQ], bf16, name="rb")
                    qeng = nc.gpsimd if (t2 + par) % 2 == 0 else nc.sync
                    qeng.dma_start(
                        rb[h_lo:h_lo + 64, :],
                        rsd.ap()[rq:rq + 1, fidx * NQ:(fidx + 1) * NQ]
                        .partition_broadcast(64).squeeze(1))
                    sl = attnT[h_lo:h_lo + 64, t2, :]
                    if not late:
                        eng = nc.gpsimd if (t2 + par) % 2 == 0 else nc.vector
                    else:
                        eng = nc.vector
                    eng.tensor_mul(sl, sl, rb[h_lo:h_lo + 64, :])

        wo_r = wo_d.rearrange("p (d j c) -> p d j c", d=4, j=HQT)

        def load_woc(j0, j1, dc, eng):
            woc = wstream.tile([P, 8, 512], bf16, name="wchunk")
            nj = j1 - j0
            eng.dma_start(
                woc[:, 0:nj, :],
                wo_r[:, dc:dc + 1, j0:j1, :].squeeze(1))
            return woc

        # wo first wave: (dc, qt) chunks of j=0..7 run at tile boundaries;
        # partial[dc*5+qt] stashes the bf16 half-sum
        wocs = {}
        tail_wocs = {}

        def wo_chunk(i):
            dc, qt = divmod(i, NQT)
            pool = woacc if qt % 2 == 0 else psS
            ps = pool.tile([P, 512], f32,
                           name="po" if qt % 2 == 0 else "ps")
            for j in range(8):
                nc.tensor.matmul(ps[:], attnT[:, j, qt * P:(qt + 1) * P],
                                 wocs[dc][:, j, :], start=(j == 0),
                                 stop=(j == 7))
            nc.vector.tensor_copy(partial[:, dc * NQT + qt, :], ps[:])

        AFTER_TILE = {
            9: [("load", 0)],
            10: [("c", 0), ("c", 1), ("c", 2), ("load", 1)],
            11: [("c", 3), ("c", 4), ("c", 5), ("load", 2)],
            12: [("c", 6), ("c", 7), ("c", 8), ("load", 3)],
            13: [("c", 9), ("c", 10), ("c", 11)],
            14: [("c", 12), ("c", 13), ("c", 14), ("c", 15), ("loadtail", 0)],
            15: [("c", 16), ("c", 17), ("c", 18), ("c", 19), ("loadtail", 1)],
        }

        for t in range(HQT):
            attn_tile(t, lambda: None)
            if t % 2 == 1:
                norm_pair(t)
            for op, i in AFTER_TILE.get(t, []):
                if op == "load":
                    wocs[i] = load_woc(0, 8, i, nc.gpsimd)
                elif op == "loadtail":
                    tail_wocs[i] = load_woc(8, 16, i, nc.gpsimd)
                else:
                    wo_chunk(i)
        if DEBUG:
            nc.sync.dma_start(
                attn_dbg, attnT[:].rearrange("p t q -> p (t q)"))
            nc.sync.dma_start(
                part_dbg, partial[:].rearrange("p c o -> p (c o)"))
            nc.sync.dma_start(rs_dbg, rs_all[:, 0, :])

        # ---- tail: j=8..15; the bf16 partial is added on the Vector engine ----
        for dc in range(4):
            woc = tail_wocs.get(dc) or load_woc(8, 16, dc, nc.sync)
            for qt in range(NQT):
                pool = woacc if qt % 2 == 0 else psS
                ps = pool.tile([P, 512], f32,
                               name="po" if qt % 2 == 0 else "ps")
                for j in range(8, 16):
                    nc.tensor.matmul(ps[:], attnT[:, j, qt * P:(qt + 1) * P],
                                     woc[:, j - 8, :], start=(j == 8),
                                     stop=(j == 15), skip_group_check=True)
                osb = work.tile([P, 512], bf16, name="osb")
                nc.vector.tensor_add(osb[:], ps[:],
                                     partial[:, dc * NQT + qt, :])
                eng = nc.scalar if (dc * NQT + qt) % 2 == 0 else nc.gpsimd
                eng.dma_start(
                    out_d[qt * P:(qt + 1) * P, dc * 512:(dc + 1) * 512],
                    osb[:])

    nc.finalize()
    _BUILD_CACHE[key] = nc
    return nc


# ---------------------------------------------------------------------------
# general device graph (fallback: multi-doc segments or key offsets)
# ---------------------------------------------------------------------------

def _build_general(NQ, NK, offs_max, causal):
    key = (NQ, NK, offs_max, causal)
    if key in _BUILD_CACHE:
        return _BUILD_CACHE[key]
    NQT, NKT = NQ // P, NK // P
    qchunks = [(c * 512, min(512, NQ - c * 512))
               for c in range((NQ + 511) // 512)]

    nc = bacc.Bacc("TRN2", target_bir_lowering=False, debug=False,
                   num_devices=N_CORES)

    xq_d = nc.dram_tensor("xq_t", [D, NQ], bf16, kind="ExternalInput").ap()
    xk_d = nc.dram_tensor("xk_t", [D, NK], bf16, kind="ExternalInput").ap()
    wq_d = nc.dram_tensor("wq_t", [D, HQ], bf16, kind="ExternalInput").ap()
    wk_d = nc.dram_tensor("wk_t", [D, HKV], bf16, kind="ExternalInput").ap()
    wv_d = nc.dram_tensor("wv_t", [D, HKV], bf16, kind="ExternalInput").ap()
    wo_d = nc.dram_tensor("wo_t", [HQ, D], bf16, kind="ExternalInput").ap()
    cosq_d = nc.dram_tensor("cos_q", [NQ, 512], bf16, kind="ExternalInput").ap()
    sinq_d = nc.dram_tensor("sin_q", [NQ, 512], bf16, kind="ExternalInput").ap()
    cosk_d = nc.dram_tensor("cos_k", [NK, 512], bf16, kind="ExternalInput").ap()
    sink_d = nc.dram_tensor("sin_k", [NK, 512], bf16, kind="ExternalInput").ap()
    mask_d = nc.dram_tensor("maskm", [NK, NQ], bf16, kind="ExternalInput").ap()
    kones_d = nc.dram_tensor("kones", [NK, 1], bf16, kind="ExternalInput").ap()
    id_d = nc.dram_tensor("ident", [P, P], bf16, kind="ExternalInput").ap()
    out_d = nc.dram_tensor("out", [NQ, HQ], f32, kind="ExternalOutput").ap()
    rsd = nc.dram_tensor("rsd", [P, NQ], bf16)

    with tile.TileContext(nc) as tc, ExitStack() as ctx:
        const = ctx.enter_context(tc.tile_pool(name="const", bufs=1))
        persist = ctx.enter_context(tc.tile_pool(name="persist", bufs=1))
        xpool = ctx.enter_context(tc.tile_pool(name="xpool", bufs=2))
        wstream = ctx.enter_context(tc.tile_pool(name="wstream", bufs=2))
        work = ctx.enter_context(tc.tile_pool(name="work", bufs=2))
        ropetab = ctx.enter_context(tc.tile_pool(name="ropetab", bufs=1))
        pmpool = ctx.enter_context(tc.tile_pool(name="pmpool", bufs=10))
        rbpool = ctx.enter_context(tc.tile_pool(name="rbpool", bufs=4))
        pp = ctx.enter_context(tc.tile_pool(name="pp", bufs=2, space="PSUM"))
        psc = ctx.enter_context(tc.tile_pool(name="psc", bufs=3, space="PSUM"))
        pv = ctx.enter_context(tc.tile_pool(name="pv", bufs=3, space="PSUM"))

        xk_sb = xpool.tile([P, DT, NK], bf16, name="xsb")
        xk_r = xk_d.rearrange("(t p) q -> p t q", p=P)
        wkc = wstream.tile([P, DT, 512], bf16, name="wchunk")
        wk_r = wk_d.rearrange("(t p) o -> p t o", p=P)
        for a, b2 in [(0, 1), (1, 2), (2, 4), (4, 8), (8, 16)]:
            nc.sync.dma_start(xk_sb[:, a:b2, :], xk_r[:, a:b2, :])
            nc.sync.dma_start(wkc[:, a:b2, :], wk_r[:, a:b2, :])

        ident = const.tile([P, P], bf16, name="ident")
        nc.sync.dma_start(ident[:], id_d)

        Qt = persist.tile([P, HQT, NQ], bf16, name="Qt")
        KtRz = persist.tile([P, KVH, NK], bf16, name="KtRz")
        Vaug = persist.tile([P, NKT, KVH, P], bf16, name="Vaug")
        attnT = persist.tile([P, HQT, NQ], bf16, name="attnT")
        mask_sb = persist.tile([P, NKT, NQ], bf16, name="mask_sb")

        nc.vector.memset(KtRz[64:128, 0:4, :], 0.0)
        nc.vector.memset(KtRz[0:64, 4:8, :], 0.0)
        nc.vector.memset(Vaug[:, :, :, HD:P], 0.0)
        kones_sb = const.tile([P, NKT], bf16, name="kones_sb")
        nc.sync.dma_start(kones_sb[:],
                          kones_d.rearrange("(t p) o -> p (t o)", p=P))
        for kt in range(NKT):
            for g in range(KVH):
                nc.vector.tensor_copy(Vaug[:, kt, g, HD:HD + 1],
                                      kones_sb[:, kt:kt + 1])

        cosk = ropetab.tile([P, NKT, 512], bf16, name="cos")
        sink = ropetab.tile([P, NKT, 512], bf16, name="sin")
        nc.sync.dma_start(cosk[:], cosk_d.rearrange("(t p) c -> p t c", p=P))
        nc.sync.dma_start(sink[:], sink_d.rearrange("(t p) c -> p t c", p=P))
        nc.sync.dma_start(mask_sb[:], mask_d.rearrange("(t p) q -> p t q", p=P))

        def rope_block(ps, cos_t, sin_t, ti):
            nat = work.tile([P, 512], f32, name="nat")
            nc.vector.tensor_copy(nat[:], ps[:])
            ro = work.tile([P, 512], f32, name="ro")
            nc.gpsimd.tensor_mul(ro[:, 0::2], nat[:, 1::2], sin_t[:, ti, 0::2])
            nc.gpsimd.tensor_mul(ro[:, 1::2], nat[:, 0::2], sin_t[:, ti, 1::2])
            tmp = work.tile([P, 512], f32, name="tmp")
            nc.vector.tensor_mul(tmp[:], nat[:], cos_t[:, ti, :])
            rot = work.tile([P, 512], bf16, name="rot")
            nc.vector.tensor_add(rot[:], ro[:], tmp[:])
            return rot

        def k_evict(ps, kt):
            rot = rope_block(ps, cosk, sink, kt)
            ks = slice(kt * P, (kt + 1) * P)
            for b in range(4):
                pst = psc.tile([P, P], bf16, name="psS")
                nc.tensor.transpose(pst[:], rot[:, b * P:(b + 1) * P], ident[:])
                half = (2 * b) // 4
                lo = half * 64
                nc.scalar.copy(KtRz[lo:lo + 64, 2 * b, ks], pst[0:64, :])
                nc.scalar.copy(KtRz[lo:lo + 64, 2 * b + 1, ks], pst[64:128, :])

        prevk = None
        for kt in range(NKT):
            ps = pp.tile([P, 512], f32, name="pj")
            for dt in range(DT):
                nc.tensor.matmul(ps[:], xk_sb[:, dt, kt * P:(kt + 1) * P],
                                 wkc[:, dt, :], start=(dt == 0),
                                 stop=(dt == DT - 1))
            if prevk is not None:
                k_evict(*prevk)
            prevk = (ps, kt)
        k_evict(*prevk)

        wvc = wstream.tile([P, DT, 512], bf16, name="wchunk")
        nc.sync.dma_start(wvc[:], wv_d.rearrange("(t p) o -> p t o", p=P))
        for kt in range(NKT):
            ps = pp.tile([P, 512], f32, name="pj")
            for dt in range(DT):
                nc.tensor.matmul(ps[:], xk_sb[:, dt, kt * P:(kt + 1) * P],
                                 wvc[:, dt, :], start=(dt == 0),
                                 stop=(dt == DT - 1))
            nc.vector.tensor_copy(Vaug[:, kt, :, 0:HD],
                                  ps[:].rearrange("p (g d) -> p g d", g=KVH))

        cosq = ropetab.tile([P, NQT, 512], bf16, name="cos")
        sinq = ropetab.tile([P, NQT, 512], bf16, name="sin")
        nc.sync.dma_start(cosq[:], cosq_d.rearrange("(t p) c -> p t c", p=P))
        nc.sync.dma_start(sinq[:], sinq_d.rearrange("(t p) c -> p t c", p=P))
        xq_sb = xpool.tile([P, DT, NQ], bf16, name="xsb")
        nc.sync.dma_start(xq_sb[:], xq_d.rearrange("(t p) q -> p t q", p=P))

        def q_evict(ps, hc, qt):
            rot = rope_block(ps, cosq, sinq, qt)
            for b in range(4):
                pst = psc.tile([P, P], bf16, name="psS")
                nc.tensor.transpose(pst[:], rot[:, b * P:(b + 1) * P], ident[:])
                dst = Qt[:, hc * 4 + b, qt * P:(qt + 1) * P]
                if b % 2 == 0:
                    nc.scalar.copy(dst, pst[:])
                else:
                    nc.vector.tensor_copy(dst, pst[:])

        prevq = None
        for hc in range(HC):
            wqc = wstream.tile([P, DT, 512], bf16, name="wchunk")
            nc.sync.dma_start(
                wqc[:],
                wq_d[:, hc * 512:(hc + 1) * 512]
                .rearrange("(t p) o -> p t o", p=P))
            for qt in range(NQT):
                ps = pp.tile([P, 512], f32, name="pj")
                for dt in range(DT):
                    nc.tensor.matmul(ps[:], xq_sb[:, dt, qt * P:(qt + 1) * P],
                                     wqc[:, dt, :], start=(dt == 0),
                                     stop=(dt == DT - 1))
                if prevq is not None:
                    q_evict(*prevq)
                prevq = (ps, hc, qt)
        q_evict(*prevq)

        rs_all = persist.tile([P, NQ], f32, name="rs_all")
        rs_rcp = persist.tile([P, NQ], bf16, name="rs_rcp")

        def norm_pass(trange, rows):
            with nc.allow_low_precision(reason="softmax denominator in bf16"):
                nc.vector.reciprocal(rs_rcp[rows], rs_all[rows])
            nc.sync.dma_start(rsd.ap()[rows, :], rs_rcp[rows])
            for t2 in trange:
                for par in range(2):
                    h_lo = par * 64
                    r = (t2 // 4) * 32 + (t2 % 4) * 2 + par
                    rb = rbpool.tile([P, NQ], bf16, name="rb")
                    nc.sync.dma_start(
                        rb[h_lo:h_lo + 64, :],
                        rsd.ap()[r:r + 1, :].partition_broadcast(64)
                        .squeeze(1))
                    sl = attnT[h_lo:h_lo + 64, t2, :]
                    nc.vector.tensor_mul(sl, sl, rb[h_lo:h_lo + 64, :])

        stash = []
        for t in range(HQT):
            groups = (t // 4, 4 + t // 4)
            for (qc, qcw) in qchunks:
                live = [kt for kt in range(NKT)
                        if kt * P <= qc + qcw - 1 + offs_max]
                psO = [pv.tile([P, 512], f32, name="pvo")[:, :qcw]
                       for _ in range(2)]
                pms = {}

                def qk_exp_mask(kt, par):
                    lo = max(0, kt * P - qc - offs_max)
                    g = groups[par]
                    psS = psc.tile([P, 512], f32, name="psS")[:, :qcw]
                    nc.tensor.matmul(
                        psS[:, lo:], KtRz[:, g, kt * P:(kt + 1) * P],
                        Qt[:, t, qc + lo:qc + qcw], start=True, stop=True)
                    if causal:
                        pm = pmpool.tile([P, 512], bf16, name="pm")[:, :qcw]
                        nc.scalar.activation(pm[:, lo:], psS[:, lo:], EXPF,
                                             bias=0.0, scale=0.125)
                        d0 = kt * P - qc
                        dlo, dhi = max(lo, d0), min(qcw, d0 + P)
                        if dlo < dhi:
                            nc.vector.tensor_mul(
                                pm[:, dlo:dhi], pm[:, dlo:dhi],
                                mask_sb[:, kt, qc + dlo:qc + dhi])
                    else:
                        pexp = pmpool.tile([P, 512], bf16, name="pexp")[:, :qcw]
                        nc.scalar.activation(pexp[:, lo:], psS[:, lo:], EXPF,
                                             bias=0.0, scale=0.125)
                        pm = pmpool.tile([P, 512], bf16, name="pm")[:, :qcw]
                        nc.vector.tensor_mul(pm[:, lo:], pexp[:, lo:],
                                             mask_sb[:, kt, qc + lo:qc + qcw])
                    return pm, lo

                def pv_mm(idx):
                    kt = live[idx]
                    for par in range(2):
                        pm, lo = pms[(idx, par)]
                        nc.tensor.matmul(
                            psO[par][:, lo:], Vaug[:, kt, groups[par], :],
                            pm[:, lo:], start=(idx == 0),
                            stop=(idx == len(live) - 1),
                            skip_group_check=True)

                for idx, kt in enumerate(live):
                    for par in range(2):
                        pms[(idx, par)] = qk_exp_mask(kt, par)
                    if idx > 0:
                        pv_mm(idx - 1)
                        del pms[(idx - 1, 0)], pms[(idx - 1, 1)]
                pv_mm(len(live) - 1)

                for par in range(2):
                    h_lo = par * 64
                    dst = attnT[h_lo:h_lo + 64, t, qc:qc + qcw]
                    nc.vector.tensor_copy(dst, psO[par][0:64, :])
                    rsum0 = work.tile([1, 512], f32, name="rsum0")[:, :qcw]
                    nc.vector.tensor_copy(rsum0, psO[par][64:65, :])
                    r = (t // 4) * 32 + (t % 4) * 2 + par
                    nc.sync.dma_start(rs_all[r:r + 1, qc:qc + qcw], rsum0)
            if t % 4 == 3:
                qi = t // 4
                norm_pass(range(qi * 4, qi * 4 + 4), slice(qi * 32, qi * 32 + 8))
            if t == 11:
                woc0 = wstream.tile([P, DT, 512], bf16, name="wchunk")
                nc.sync.dma_start(
                    woc0[:], wo_d[:, 0:512].rearrange("(t p) o -> p t o", p=P))
                for qt0 in range(2):
                    ps0 = pp.tile([P, 512], f32, name="pj")
                    for j in range(12):
                        nc.tensor.matmul(ps0[:],
                                         attnT[:, j, qt0 * P:(qt0 + 1) * P],
                                         woc0[:, j, :], start=(j == 0),
                                         stop=False)
                    stash.append((ps0, woc0, 0, qt0))

        def wo_finish(ps, woc, dc, qt, jlo):
            for j in range(jlo, HQT):
                nc.tensor.matmul(ps[:], attnT[:, j, qt * P:(qt + 1) * P],
                                 woc[:, j, :], start=(j == 0),
                                 stop=(j == HQT - 1))
            osb = work.tile([P, 512], f32, name="osb")
            nc.vector.tensor_copy(osb[:], ps[:])
            nc.sync.dma_start(
                out_d[qt * P:(qt + 1) * P, dc * 512:(dc + 1) * 512], osb[:])

        for args in stash:
            wo_finish(*args, 12)
        for dc in range(4):
            if dc == 0:
                woc = stash[0][1]
            else:
                woc = wstream.tile([P, DT, 512], bf16, name="wchunk")
                nc.sync.dma_start(
                    woc[:], wo_d[:, dc * 512:(dc + 1) * 512]
                    .rearrange("(t p) o -> p t o", p=P))
            for qt in range(NQT):
                if dc == 0 and qt < 2:
                    continue
                ps = pp.tile([P, 512], f32, name="pj")
                wo_finish(ps, woc, dc, qt, 0)

    nc.finalize()
    _BUILD_CACHE[key] = nc
    return nc


# ---------------------------------------------------------------------------
# entry point
# ---------------------------------------------------------------------------

def _prepare(x, freqs_cis, sequence_id, wq, wk, wv, wo):
    """Plan jobs, build the device graph, and assemble per-core inputs.
    Returns (nc, in_maps, jobs)."""
    x = np.asarray(x, dtype=np.float32)
    freqs_cis = np.asarray(freqs_cis, dtype=np.float32)
    sequence_id = np.asarray(sequence_id)

    jobs = _plan_jobs(sequence_id)
    NQ = _round_up(max(max(j[2] for j in jobs), 1), P)
    NK = _round_up(max(max(j[1] + j[2] - j[3] for j in jobs), 1), P)
    offs_max = max(j[1] - j[3] for j in jobs)

    def single_doc(j):
        b, qs, ql, ks = j
        if ql == 0:
            return True
        seg = np.asarray(sequence_id[b])[ks:qs + ql]
        return bool((seg == seg[0]).all())

    causal = offs_max == 0 and all(single_doc(j) for j in jobs)

    cos_tab = freqs_cis[:, :, 0].astype(np.float32)
    sin_tab = freqs_cis[:, :, 1].astype(np.float32)
    wq_t = _permute_wq(
        np.ascontiguousarray(np.asarray(wq, np.float32).T)).astype(bf)
    wk_t = np.ascontiguousarray(np.asarray(wk, np.float32).T).astype(bf)
    wv_t = np.ascontiguousarray(np.asarray(wv, np.float32).T).astype(bf)
    wo_t = _permute_wo(
        np.ascontiguousarray(np.asarray(wo, np.float32).T)).astype(bf)
    id16 = np.eye(P, dtype=bf)

    fast = causal and NQ == 640 and NK == NQ
    if fast:
        # p-major layouts: long contiguous DMA lines per partition
        wq_t = np.ascontiguousarray(
            wq_t.reshape(DT, P, HC, 512).transpose(1, 2, 0, 3)
            .reshape(P, HC * DT * 512))
        wk_t = _pmajor(wk_t, P, HKV)
        wv_t = _pmajor(wv_t, P, HKV)
        wo_t = np.ascontiguousarray(
            wo_t.reshape(HQT, P, 4, 512).transpose(1, 2, 0, 3)
            .reshape(P, 4 * HQT * 512))
    in_maps = []
    for job in jobs:
        if fast:
            p = _core_inputs_causal(job, NQ, x, sequence_id, cos_tab, sin_tab)
        else:
            p = _core_inputs_general(job, NQ, NK, x, sequence_id,
                                     cos_tab, sin_tab)
        p.update({"wq_t": wq_t, "wk_t": wk_t, "wv_t": wv_t, "wo_t": wo_t,
                  "ident": id16})
        in_maps.append(p)

    if fast:
        nc = _build_causal(NQ)
    else:
        nc = _build_general(NQ, NK, offs_max, causal)
    return nc, in_maps, jobs


def kernel(x, freqs_cis, sequence_id, wq, wk, wv, wo):
    nc, in_maps, jobs = _prepare(x, freqs_cis, sequence_id, wq, wk, wv, wo)
    res = run_bass_kernel_spmd(nc, in_maps, core_ids=list(range(N_CORES)))

    full = np.zeros((BS, S, HQ), dtype=np.float32)
    for job, r in zip(jobs, res.results):
        b, qs, ql, ks = job
        if ql > 0:
            full[b, qs:qs + ql] = np.asarray(r["out"][:ql], dtype=np.float32)
    return full

